# revision 3
# baseline (speedup 1.0000x reference)
"""Trainium2 Bass kernel for nn_GammaNeuronNet (conductance-based neuron network).

Strategy
--------
N=4096 neurons, 300 sequential timesteps. Per step, three matvecs against two
constant 4096x4096 matrices (G_syn used twice, G_gap once), then an
elementwise state update of (V, s).

* Row-partition G_syn/G_gap across the 8 cores (512 rows each). Both shards
  are cast to bf16 and kept SBUF-resident for the whole kernel (8 MB/core),
  so HBM is only touched once for the matrices.
* The two matrices are merged along the contraction axis: one accumulation
  of 64 k-tiles computes   col0 = G_syn @ s   (zeros for the G_gap half)
  and                      col1 = G_syn @ (s*E_syn) + G_gap @ V
  which is all the reference needs (int_syn and int_gap only appear summed).
* Matmuls are x-stationary: lhsT = [s|sE / 0|V] tile [128,2], rhs = G^T tile
  [128,512] streamed, PSUM out [2,512].  PE-transposes convert [2,512] into
  the [128, 4] per-row layout used by the elementwise update.
* The elementwise update uses the identity
      V_inf - V = dV / denom   =>   vstep = dV * min(dt, 1/denom)
  which is mathematically identical to the reference's clip().
* Per step the updated (V,s) slices (4 KB) are exchanged with an 8-core
  AllGather; the gathered state directly lands in the k-layout needed to
  rebuild the stationary x-tiles for the next step.

Global state layout ("L2"): neuron n lives at state[n//32, n%32] (V) and
state[n//32, 32 + n%32] (s) of a [128, 64] f32 tile.  k-tile t of the
matmul contracts over {k : k % 32 == t}, with partition p holding k=32p+t.
"""

import os
import numpy as np
import ml_dtypes

N = 4096
NCORES = 8
ROWS = N // NCORES            # 512 matrix rows per core
MT = ROWS // 128              # 4 m-tiles of 128 rows
KTM = N // 128                # 32 k-tiles per matrix
KT = 2 * KTM                  # 64 merged k-tiles (G_syn then G_gap)
BETA, V_TH, A_R, A_D = 0.125, -15.0, 1.0, 5.0

_cache = {}
last_results = None


def _n_steps(timestep, runtime):
    # replicate the reference's float-accumulation loop exactly
    t, n = 0.0, 0
    while t < runtime:
        t += timestep
        n += 1
    return n


def _build(n_steps: int, dt: float):
    import concourse.bacc as bacc
    import concourse.mybir as mybir
    import concourse.tile as tile
    from concourse import masks

    f32 = mybir.dt.float32
    bf16 = mybir.dt.bfloat16

    nc = bacc.Bacc("TRN2", target_bir_lowering=False, debug=False,
                   num_devices=NCORES)

    w_d = nc.dram_tensor("w_in", [128, KT * ROWS], bf16, kind="ExternalInput")
    state0_d = nc.dram_tensor("state0_in", [128, 64], f32, kind="ExternalInput")
    vs0_d = nc.dram_tensor("vs0_in", [128, 2 * MT], f32, kind="ExternalInput")
    c0_d = nc.dram_tensor("c0_in", [128, MT], f32, kind="ExternalInput")
    gle_d = nc.dram_tensor("gle_in", [128, MT], f32, kind="ExternalInput")
    esyn_d = nc.dram_tensor("esyn_in", [128, 32], f32, kind="ExternalInput")
    vout_d = nc.dram_tensor("v_out", [128, MT], f32, kind="ExternalOutput")

    rg = [list(range(NCORES))]
    Sigmoid = mybir.ActivationFunctionType.Sigmoid
    Copy = mybir.ActivationFunctionType.Copy

    ar_dt = float(A_R) * dt              # u = ar_dt * sigmoid(...)
    c1 = 1.0 - float(A_D) * dt           # s_new = s*(c1 - u) + u
    sig_scale = float(BETA)
    sig_bias = -float(BETA) * float(V_TH)

    with tile.TileContext(nc) as tc:
        with (
            tc.tile_pool(name="const", bufs=1) as constp,
            tc.tile_pool(name="wpool", bufs=1) as wp,
            tc.tile_pool(name="xwpool", bufs=1) as xwp,
            tc.tile_pool(name="state", bufs=2) as statep,
            tc.tile_pool(name="vs", bufs=2) as vsp,
            tc.tile_pool(name="ew", bufs=2) as ewp,
            tc.tile_pool(name="csb", bufs=2) as csbp,
            tc.tile_pool(name="mm", bufs=2, space="PSUM") as mmp,
            tc.tile_pool(name="pe", bufs=2, space="PSUM") as pep,
            tc.tile_pool(name="ttp", bufs=2, space="PSUM") as ttp,
            tc.tile_pool(name="dram", bufs=2, space="DRAM") as dramp,
        ):
            w_sb = wp.tile([128, KT * ROWS], bf16)
            nc.sync.dma_start(w_sb[:], w_d[:])
            c0_sb = constp.tile([128, MT], f32)
            nc.sync.dma_start(c0_sb[:], c0_d[:])
            gle_sb = constp.tile([128, MT], f32)
            nc.sync.dma_start(gle_sb[:], gle_d[:])
            esyn_sb = constp.tile([128, 32], f32)
            nc.sync.dma_start(esyn_sb[:], esyn_d[:])
            ident = constp.tile([128, 128], f32)
            masks.make_identity(nc, ident[:])
            sigb_sb = constp.tile([128, 1], f32)
            nc.vector.memset(sigb_sb[:], sig_bias)

            xw = xwp.tile([128, KT * 2], bf16)
            nc.vector.memset(xw[:], 0.0)

            state = statep.tile([128, 64], f32, tag="state")
            nc.sync.dma_start(state[:], state0_d[:])
            vs = vsp.tile([128, 2 * MT], f32, tag="vs")
            nc.sync.dma_start(vs[:], vs0_d[:])

            def build_xw(state_t):
                xw3 = xw[:].rearrange("p (k j) -> p k j", j=2)
                s_view = state_t[:, 32:64]
                v_view = state_t[:, 0:32]
                nc.vector.tensor_copy(xw3[:, 0:KTM, 0], s_view)
                nc.vector.tensor_mul(xw3[:, 0:KTM, 1], s_view, esyn_sb[:])
                nc.vector.tensor_copy(xw3[:, KTM:KT, 1], v_view)

            build_xw(state)

            for i in range(n_steps):
                last = i == n_steps - 1
                # ---- matvecs: 64 accumulating matmuls, out [2, 512]
                mm = mmp.tile([2, ROWS], f32, tag="mm")
                xw3 = xw[:].rearrange("p (k j) -> p k j", j=2)
                for kt in range(KT):
                    nc.tensor.matmul(
                        mm[:, :],
                        xw3[:, kt, :],
                        w_sb[:, kt * ROWS:(kt + 1) * ROWS],
                        start=(kt == 0),
                        stop=(kt == KT - 1),
                    )

                # ---- PSUM [2,512] -> SBUF, then 4 PE-transposes -> [128, (mt,j)]
                cs_sb = csbp.tile([2, ROWS], f32, tag="cs")
                nc.vector.tensor_copy(cs_sb[:], mm[:])
                pe_ps = pep.tile([128, 2 * MT], f32, tag="pe")
                for mt in range(MT):
                    nc.tensor.transpose(
                        pe_ps[:, 2 * mt:2 * mt + 2],
                        cs_sb[:, mt * 128:(mt + 1) * 128],
                        ident[:2, :2],
                    )
                pe3 = pe_ps[:].rearrange("p (m j) -> p m j", j=2)
                cs = pe3[:, :, 0]       # co_syn          [128, MT]
                ints = pe3[:, :, 1]     # int_syn+int_gap [128, MT]
                V = vs[:, 0:MT]
                S = vs[:, MT:2 * MT]

                den = ewp.tile([128, MT], f32, tag="den")
                num = ewp.tile([128, MT], f32, tag="num")
                dV = ewp.tile([128, MT], f32, tag="dv")
                r = ewp.tile([128, MT], f32, tag="r")
                sg = ewp.tile([128, MT], f32, tag="sg")
                u = ewp.tile([128, MT], f32, tag="u")
                w_ = ewp.tile([128, MT], f32, tag="w")
                vs_new = vsp.tile([128, 2 * MT], f32, tag="vs")

                nc.vector.tensor_add(den[:], cs, c0_sb[:])
                nc.vector.tensor_add(num[:], ints, gle_sb[:])
                nc.vector.tensor_mul(dV[:], V, den[:])
                nc.vector.tensor_sub(dV[:], num[:], dV[:])       # dV = num - V*den
                nc.vector.reciprocal(r[:], den[:])
                nc.vector.tensor_scalar_min(r[:], r[:], dt)      # min(1/den, dt)
                nc.vector.tensor_mul(dV[:], dV[:], r[:])         # vstep
                nc.vector.tensor_add(vs_new[:, 0:MT], V, dV[:])
                nc.scalar.activation(sg[:], V, Sigmoid, bias=sigb_sb[:, 0:1], scale=sig_scale)
                nc.scalar.activation(u[:], sg[:], Copy, bias=0.0, scale=ar_dt)
                nc.scalar.activation(w_[:], u[:], Copy, bias=c1, scale=-1.0)
                nc.vector.tensor_mul(sg[:], S, w_[:])            # s*(c1-u), reuse sg
                nc.vector.tensor_add(vs_new[:, MT:2 * MT], sg[:], u[:])

                vs = vs_new
                if last:
                    nc.sync.dma_start(vout_d[:], vs_new[:, 0:MT])
                    break

                # ---- exchange: transpose [128,8] -> [8,128], DMA to DRAM slice,
                #      AllGather, read back full state, rebuild x-tiles
                tt_ps = ttp.tile([2 * MT, 128], f32, tag="tt")
                nc.tensor.transpose(tt_ps[:], vs_new[:], ident[:128, :128])
                tt_sb = csbp.tile([2 * MT, 128], f32, tag="ttsb")
                nc.vector.tensor_copy(tt_sb[:], tt_ps[:])

                ccin = dramp.tile([16, 64], f32, tag="ccin")
                ccout = dramp.tile([128, 64], f32, tag="ccout")
                cc3 = ccin[:].rearrange("(r b) c -> r b c", b=4)
                nc.sync.dma_start(
                    cc3[:, :, 0:32],
                    tt_sb[0:MT, :].rearrange("r (b pp) -> r b pp", pp=32),
                )
                nc.sync.dma_start(
                    cc3[:, :, 32:64],
                    tt_sb[MT:2 * MT, :].rearrange("r (b pp) -> r b pp", pp=32),
                )
                nc.gpsimd.collective_compute(
                    "AllGather",
                    mybir.AluOpType.bypass,
                    replica_groups=rg,
                    ins=[ccin[:].opt()],
                    outs=[ccout[:].opt()],
                )
                state = statep.tile([128, 64], f32, tag="state")
                nc.sync.dma_start(state[:], ccout[:])
                build_xw(state)

    nc.compile()
    return nc


def _prep(input_V, G_leak, E_leak, G_syn, E_syn, G_gap):
    iv = np.asarray(input_V, np.float32).reshape(-1)
    G_leak = np.asarray(G_leak, np.float32)
    E_leak = np.asarray(E_leak, np.float32)
    G_syn = np.asarray(G_syn, np.float32)
    E_syn = np.asarray(E_syn, np.float32)
    G_gap = np.asarray(G_gap, np.float32)
    in_len = iv.shape[0]

    in_avg = np.float32(iv.mean(dtype=np.float32))
    V0 = np.concatenate([iv, np.full(N - in_len, in_avg, np.float32)])
    x = (BETA * (V0 - V_TH)).astype(np.float32)
    sig = (1.0 / (1.0 + np.exp(-x, dtype=np.float32))).astype(np.float32)
    s0 = (A_R * sig / (A_R * sig + A_D)).astype(np.float32)
    co_gap = G_gap.sum(axis=1, dtype=np.float32)
    c0_full = (G_leak + co_gap).astype(np.float32)
    gle_full = (G_leak * E_leak).astype(np.float32)

    Gs16 = G_syn.astype(ml_dtypes.bfloat16)
    Gg16 = G_gap.astype(ml_dtypes.bfloat16)

    state0 = np.ascontiguousarray(
        np.concatenate([V0.reshape(128, 32), s0.reshape(128, 32)], axis=1)
    )
    esyn_l2 = np.ascontiguousarray(E_syn.reshape(128, 32))

    in_maps = []
    for c in range(NCORES):
        rows = slice(c * ROWS, (c + 1) * ROWS)
        A_s = Gs16[rows, :].reshape(ROWS, 128, 32)   # [n, p, t], k = 32p + t
        A_g = Gg16[rows, :].reshape(ROWS, 128, 32)
        Ws = np.transpose(A_s, (1, 2, 0))            # [p, t, n]
        Wg = np.transpose(A_g, (1, 2, 0))
        W = np.ascontiguousarray(
            np.concatenate([Ws, Wg], axis=1)
        ).reshape(128, KT * ROWS)
        vs0 = np.ascontiguousarray(
            np.concatenate(
                [V0[rows].reshape(MT, 128).T, s0[rows].reshape(MT, 128).T], axis=1
            )
        )
        c0 = np.ascontiguousarray(c0_full[rows].reshape(MT, 128).T)
        gle = np.ascontiguousarray(gle_full[rows].reshape(MT, 128).T)
        in_maps.append({
            "w_in": W,
            "state0_in": state0,
            "vs0_in": vs0,
            "c0_in": c0,
            "gle_in": gle,
            "esyn_in": esyn_l2,
        })
    return in_maps, in_len


def kernel(input_V, G_leak, E_leak, G_syn, E_syn, G_gap, timestep, runtime):
    global last_results
    from concourse.bass_utils import run_bass_kernel_spmd

    dt = float(np.asarray(timestep))
    rt = float(np.asarray(runtime))
    n_steps = _n_steps(dt, rt)

    key = (n_steps, dt)
    if key not in _cache:
        _cache[key] = _build(n_steps, dt)
    nc = _cache[key]

    in_maps, in_len = _prep(input_V, G_leak, E_leak, G_syn, E_syn, G_gap)
    trace = os.environ.get("GAMMA_TRACE", "0") == "1"
    res = run_bass_kernel_spmd(
        nc, in_maps, core_ids=list(range(NCORES)), trace=trace
    )
    last_results = res

    V = np.concatenate(
        [np.asarray(res.results[c]["v_out"]).T.reshape(ROWS) for c in range(NCORES)]
    ).astype(np.float32)
    V[in_len:] = 0.0
    return V
